# revision 20
# baseline (speedup 1.0000x reference)
"""Trainium2 Bass kernel for the attention-scoring module:

    q = query @ Wq.T + bq                               # (B, D)
    ref[b,d,k] = sum_e enc[k,b,e] * Wref[d,e] + bref[d]
    u[b,k] = sum_d v[d] * tanh(ref[b,d,k] + q[b,d])
    out = 10 * tanh(u)                                  # (B, K)

Data-parallel over batch: core c owns b in [32c, 32c+32).

Per-core dataflow (all big tensors bf16, f32 accumulation):
  - host pre-transposes enc to (E, b*K+k) so the contraction dim E lands
    on SBUF partitions with dense DMA.
  - main matmuls: psum[d(128), n(512)] += WrefT_chunk.T @ encT_chunk
  - bias (bref+bq+q_raw[b])[d] is per-partition in this layout -> folded
    into the ScalarE tanh activation for free.
  - the v-weighted d-reduction is a second-level matmul with stationary
    v (128,1): strips (1, 512) for the four k-blocks of one b land at
    partitions {0,32,64,96} of one PSUM bank via tile_position col
    groups (bank pre-zeroed, accumulation via start=False).
  - final 10*tanh(u) runs on the whole strip window (junk rows are
    free); the per-b output DMA plucks rows {0,32,64,96} with a
    stepped-partition access pattern.
"""

import os
import sys

import numpy as np

for _p in ("/opt/trn_rl_repo", "/opt/pypackages"):
    if _p not in sys.path:
        sys.path.append(_p)

import ml_dtypes

E = 256
D = 256
K = 2048
B = 256
NCORES = 8
BL = B // NCORES          # 32 batch rows per core
N = BL * K                # 65536 flattened (b, k) per core
SLAB_B = 4                # b-rows per enc DMA slab
SLAB_N = SLAB_B * K       # 8192
C_CLIP = 10.0

_compiled = None
last_exec_time_ns = None
last_results = None


def _build():
    from concourse import bacc, bass, tile

    mybir = bass.mybir
    dt = mybir.dt
    AF = mybir.ActivationFunctionType

    nc = bacc.Bacc("TRN2", target_bir_lowering=False, debug=False,
                   num_devices=NCORES)

    enc_t = nc.declare_dram_parameter("enc_t", [E, N], dt.bfloat16, isOutput=False)
    wref_t = nc.declare_dram_parameter("wref_t", [4 * 128, 128], dt.bfloat16, isOutput=False)
    wq_t = nc.declare_dram_parameter("wq_t", [4 * 128, 128], dt.float32, isOutput=False)
    query_t = nc.declare_dram_parameter("query_t", [E, BL], dt.float32, isOutput=False)
    cbias_t = nc.declare_dram_parameter("cbias_t", [E, 1], dt.float32, isOutput=False)
    v_t = nc.declare_dram_parameter("v_t", [E, 1], dt.bfloat16, isOutput=False)
    out_p = nc.declare_dram_parameter("out", [BL, K], dt.float32, isOutput=True)

    with tile.TileContext(nc) as tc:
        with (
            tc.tile_pool(name="const", bufs=1) as constp,
            tc.tile_pool(name="enc", bufs=3) as encp,
            tc.tile_pool(name="tt", bufs=6) as tp,
            tc.tile_pool(name="tail", bufs=2) as tailp,
            tc.tile_pool(name="psum_m", bufs=3, space="PSUM") as pmp,
            tc.tile_pool(name="psum_s", bufs=2, space="PSUM") as psp,
        ):
            # ---- enc slab loading (slab 0 first, split per b-row so the
            # first matmuls start as early as possible) ----
            def load_slab(s):
                # split per b-row, ec-interleaved: the first matmuls of the
                # slab wait only on the first two pieces
                tiles = [encp.tile([128, SLAB_N], dt.bfloat16, tag=f"enc{ec}",
                                   name=f"enc{ec}_s{s}")
                         for ec in range(2)]
                for q in range(SLAB_B):
                    for ec in range(2):
                        nc.sync.dma_start(
                            tiles[ec][:, q * K:(q + 1) * K],
                            enc_t[ec * 128:(ec + 1) * 128,
                                  s * SLAB_N + q * K:s * SLAB_N + (q + 1) * K])
                return tiles

            # ---- constants (weights first so the first matmuls are not
            # stuck behind bulk enc traffic in the DMA queue) ----
            wref_sb = constp.tile([128, 512], dt.bfloat16)   # [:, (ec*2+dc)*128 + d]
            wq_sb = constp.tile([128, 512], dt.float32)
            query_sb = constp.tile([128, 2 * BL], dt.float32)  # [:, ec*32 + b]
            cbias_sb = constp.tile([128, 2], dt.float32)
            v_sb = constp.tile([128, 2], dt.bfloat16)
            bias_sb = constp.tile([128, 2 * BL], dt.float32)   # [:, dc*32 + b]

            for c in range(4):
                nc.sync.dma_start(wref_sb[:, c * 128:(c + 1) * 128],
                                  wref_t[c * 128:(c + 1) * 128, :])
                nc.sync.dma_start(wq_sb[:, c * 128:(c + 1) * 128],
                                  wq_t[c * 128:(c + 1) * 128, :])
            for ec in range(2):
                nc.sync.dma_start(query_sb[:, ec * BL:(ec + 1) * BL],
                                  query_t[ec * 128:(ec + 1) * 128, :])
            for dc in range(2):
                nc.sync.dma_start(cbias_sb[:, dc:dc + 1],
                                  cbias_t[dc * 128:(dc + 1) * 128, :])
                nc.sync.dma_start(v_sb[:, dc:dc + 1],
                                  v_t[dc * 128:(dc + 1) * 128, :])

            slabs = {0: load_slab(0), 1: load_slab(1)}

            # ---- q_rawT = (query @ Wq.T).T per d-chunk, + (bref + bq) ----
            for dc in range(2):
                qps = psp.tile([128, BL], dt.float32, tag="st")
                for ec in range(2):
                    nc.tensor.matmul(
                        qps[:],
                        wq_sb[:, (ec * 2 + dc) * 128:(ec * 2 + dc + 1) * 128],
                        query_sb[:, ec * BL:(ec + 1) * BL],
                        start=(ec == 0), stop=(ec == 1),
                    )
                nc.vector.tensor_scalar_add(bias_sb[:, dc * BL:(dc + 1) * BL],
                                            qps[:], cbias_sb[:, dc:dc + 1])

            # ---- main loop, v-matmuls software-pipelined one round behind
            # the main matmuls so they never stall TensorE on ScalarE ----
            def emit_epilogue(st4, tts, b, kp):
                for kb in range(2):
                    jj = kp * 2 + kb
                    for dc in range(2):
                        # start=True clears has_written per element, so the
                        # sibling strips in the same bank are unaffected
                        nc.tensor.matmul(
                            st4[32 * jj:32 * jj + 1, :],
                            v_sb[:, dc:dc + 1],
                            tts[dc][:, kb * 512:(kb + 1) * 512],
                            start=(dc == 0), stop=(dc == 1),
                            skip_group_check=True,
                            tile_position=(0, 32 * jj),
                        )
                if kp == 1:
                    # out[b, :] = 10 * tanh(strips); junk rows are free
                    t5 = tailp.tile([128, 512], dt.float32, tag="t5")
                    nc.scalar.activation(t5[:], st4[:], AF.Tanh)
                    o5 = tailp.tile([128, 512], dt.float32, tag="o5")
                    nc.vector.tensor_scalar_mul(o5[:], t5[:], C_CLIP)
                    nc.sync.dma_start(out_p[b:b + 1, :], o5[0:128:32, :])

            pend = None
            NSLAB = N // SLAB_N
            for s in range(NSLAB):                  # 8 slabs of 4 b-rows
                if s + 2 < NSLAB:
                    slabs[s + 2] = load_slab(s + 2)
                cur_slab = slabs.pop(s)
                for b_in in range(SLAB_B):
                    b = SLAB_B * s + b_in
                    st4 = psp.tile([128, 512], dt.float32, tag="st")
                    for kp in range(2):             # two 1024-wide n groups
                        tts = []
                        for dc in range(2):
                            psd = pmp.tile([128, 1024], dt.float32, tag="psd")
                            # ec outer / kb inner: consecutive matmuls share
                            # the stationary operand -> fewer weight loads
                            for ec in range(2):
                                for kb in range(2):
                                    nseg = b_in * K + kp * 1024 + kb * 512
                                    nc.tensor.matmul(
                                        psd[:, kb * 512:(kb + 1) * 512],
                                        wref_sb[:, (ec * 2 + dc) * 128:(ec * 2 + dc + 1) * 128],
                                        cur_slab[ec][:, nseg:nseg + 512],
                                        start=(ec == 0), stop=(ec == 1),
                                        skip_group_check=True,
                                    )
                            ttile = tp.tile([128, 1024], dt.bfloat16, tag="tt")
                            nc.scalar.activation(
                                ttile[:], psd[:], AF.Tanh,
                                bias=bias_sb[:, dc * BL + b:dc * BL + b + 1],
                                scale=1.0)
                            tts.append(ttile)
                        if pend is not None:
                            emit_epilogue(*pend)
                        pend = (st4, tts, b, kp)
            emit_epilogue(*pend)

    nc.compile()
    return nc


def _prep_inputs(encoder_output, query, Wq, bq, Wref, bref, v):
    bf16 = ml_dtypes.bfloat16
    # (K, B, E) -> (E, B, K), bf16
    enc_bf = np.asarray(encoder_output, np.float32).astype(bf16)
    encT = enc_bf.transpose(2, 1, 0)                   # (E, B, K) view

    def chunk4(w):                                     # (E, D) -> (4*128, 128)
        return np.ascontiguousarray(
            w.reshape(2, 128, 2, 128).transpose(0, 2, 1, 3).reshape(512, 128))

    wrefT = chunk4(np.asarray(Wref, np.float32).T).astype(bf16)
    wqT = chunk4(np.asarray(Wq, np.float32).T)
    cbias = (np.asarray(bref, np.float32) + np.asarray(bq, np.float32)).reshape(E, 1)
    v_col = np.asarray(v, np.float32).astype(bf16).reshape(E, 1)
    queryT = np.ascontiguousarray(np.asarray(query, np.float32).T)  # (E, B)

    in_maps = []
    for c in range(NCORES):
        enc_c = np.ascontiguousarray(encT[:, c * BL:(c + 1) * BL, :]).reshape(E, N)
        in_maps.append({
            "enc_t": enc_c,
            "wref_t": wrefT,
            "wq_t": wqT,
            "query_t": np.ascontiguousarray(queryT[:, c * BL:(c + 1) * BL]),
            "cbias_t": cbias,
            "v_t": v_col,
        })
    return in_maps


def kernel(**inputs):
    global _compiled, last_exec_time_ns, last_results
    from concourse import bass_utils

    if _compiled is None:
        _compiled = _build()
    nc = _compiled

    in_maps = _prep_inputs(**inputs)
    res = bass_utils.run_bass_kernel_spmd(nc, in_maps, core_ids=list(range(NCORES)))
    last_exec_time_ns = res.exec_time_ns
    last_results = res
    out = np.concatenate([r["out"] for r in res.results], axis=0)
    return out


# revision 25
# speedup vs baseline: 1.0129x; 1.0129x over previous
"""Trainium2 Bass kernel for the attention-scoring module:

    q = query @ Wq.T + bq                               # (B, D)
    ref[b,d,k] = sum_e enc[k,b,e] * Wref[d,e] + bref[d]
    u[b,k] = sum_d v[d] * tanh(ref[b,d,k] + q[b,d])
    out = 10 * tanh(u)                                  # (B, K)

Data-parallel over batch: core c owns b in [32c, 32c+32).

Per-core dataflow (all big tensors bf16, f32 accumulation):
  - host pre-transposes enc to (E, b*K+k) so the contraction dim E lands
    on SBUF partitions with dense DMA.
  - main matmuls: psum[d(128), n(512)] += WrefT_chunk.T @ encT_chunk
  - bias (bref+bq+q_raw[b])[d] is per-partition in this layout -> folded
    into the ScalarE tanh activation for free.
  - the v-weighted d-reduction is a second-level matmul with stationary
    v (128,1): strips (1, 512) for the four k-blocks of one b land at
    partitions {0,32,64,96} of one PSUM bank via tile_position col
    groups (bank pre-zeroed, accumulation via start=False).
  - final 10*tanh(u) runs on the whole strip window (junk rows are
    free); the per-b output DMA plucks rows {0,32,64,96} with a
    stepped-partition access pattern.
"""

import os
import sys

import numpy as np

for _p in ("/opt/trn_rl_repo", "/opt/pypackages"):
    if _p not in sys.path:
        sys.path.append(_p)

import ml_dtypes

E = 256
D = 256
K = 2048
B = 256
NCORES = 8
BL = B // NCORES          # 32 batch rows per core
N = BL * K                # 65536 flattened (b, k) per core
SLAB_B = 4                # b-rows per enc DMA slab
SLAB_N = SLAB_B * K       # 8192
C_CLIP = 10.0

_compiled = None
last_exec_time_ns = None
last_results = None


def _build():
    from concourse import bacc, bass, tile

    mybir = bass.mybir
    dt = mybir.dt
    AF = mybir.ActivationFunctionType

    nc = bacc.Bacc("TRN2", target_bir_lowering=False, debug=False,
                   num_devices=NCORES)

    enc_t = nc.declare_dram_parameter("enc_t", [E, N], dt.bfloat16, isOutput=False)
    wref_t = nc.declare_dram_parameter("wref_t", [4 * 128, 128], dt.bfloat16, isOutput=False)
    wq_t = nc.declare_dram_parameter("wq_t", [4 * 128, 128], dt.float32, isOutput=False)
    query_t = nc.declare_dram_parameter("query_t", [E, BL], dt.float32, isOutput=False)
    cbias_t = nc.declare_dram_parameter("cbias_t", [E, 1], dt.float32, isOutput=False)
    v_t = nc.declare_dram_parameter("v_t", [E, 1], dt.bfloat16, isOutput=False)
    out_p = nc.declare_dram_parameter("out", [BL, K], dt.float32, isOutput=True)

    with tile.TileContext(nc) as tc:
        with (
            tc.tile_pool(name="const", bufs=1) as constp,
            tc.tile_pool(name="enc", bufs=3) as encp,
            tc.tile_pool(name="tt", bufs=6) as tp,
            tc.tile_pool(name="tail", bufs=2) as tailp,
            tc.tile_pool(name="psum_m", bufs=3, space="PSUM") as pmp,
            tc.tile_pool(name="psum_s", bufs=2, space="PSUM") as psp,
        ):
            # ---- enc slab loading (slab 0 first, split per b-row so the
            # first matmuls start as early as possible) ----
            def load_slab(s, pieces):
                # ec-interleaved pieces: the first matmuls of the slab wait
                # only on the first piece of each e-chunk
                tiles = [encp.tile([128, SLAB_N], dt.bfloat16, tag=f"enc{ec}",
                                   name=f"enc{ec}_s{s}")
                         for ec in range(2)]
                w = SLAB_N // pieces
                for q in range(pieces):
                    for ec in range(2):
                        nc.sync.dma_start(
                            tiles[ec][:, q * w:(q + 1) * w],
                            enc_t[ec * 128:(ec + 1) * 128,
                                  s * SLAB_N + q * w:s * SLAB_N + (q + 1) * w])
                return tiles

            # ---- constants (weights first so the first matmuls are not
            # stuck behind bulk enc traffic in the DMA queue) ----
            wref_sb = constp.tile([128, 512], dt.bfloat16)   # [:, (ec*2+dc)*128 + d]
            wq_sb = constp.tile([128, 512], dt.float32)
            query_sb = constp.tile([128, 2 * BL], dt.float32)  # [:, ec*32 + b]
            cbias_sb = constp.tile([128, 2], dt.float32)
            v_sb = constp.tile([128, 2], dt.bfloat16)
            bias_sb = constp.tile([128, 2 * BL], dt.float32)   # [:, dc*32 + b]

            for c in range(4):
                nc.sync.dma_start(wq_sb[:, c * 128:(c + 1) * 128],
                                  wq_t[c * 128:(c + 1) * 128, :])
            for ec in range(2):
                nc.sync.dma_start(query_sb[:, ec * BL:(ec + 1) * BL],
                                  query_t[ec * 128:(ec + 1) * 128, :])
            for c in range(4):
                nc.sync.dma_start(wref_sb[:, c * 128:(c + 1) * 128],
                                  wref_t[c * 128:(c + 1) * 128, :])
            for dc in range(2):
                nc.sync.dma_start(cbias_sb[:, dc:dc + 1],
                                  cbias_t[dc * 128:(dc + 1) * 128, :])
                nc.sync.dma_start(v_sb[:, dc:dc + 1],
                                  v_t[dc * 128:(dc + 1) * 128, :])

            cur_slab = load_slab(0, pieces=SLAB_B)

            # ---- q_rawT = (query @ Wq.T).T per d-chunk, + (bref + bq) ----
            for dc in range(2):
                qps = psp.tile([128, BL], dt.float32, tag="st")
                for ec in range(2):
                    nc.tensor.matmul(
                        qps[:],
                        wq_sb[:, (ec * 2 + dc) * 128:(ec * 2 + dc + 1) * 128],
                        query_sb[:, ec * BL:(ec + 1) * BL],
                        start=(ec == 0), stop=(ec == 1),
                    )
                nc.vector.tensor_scalar_add(bias_sb[:, dc * BL:(dc + 1) * BL],
                                            qps[:], cbias_sb[:, dc:dc + 1])

            # ---- main loop, v-matmuls software-pipelined one round behind
            # the main matmuls so they never stall TensorE on ScalarE ----
            def emit_epilogue(st4, tts, b, kp):
                for kb in range(2):
                    jj = kp * 2 + kb
                    for dc in range(2):
                        # start=True clears has_written per element, so the
                        # sibling strips in the same bank are unaffected
                        nc.tensor.matmul(
                            st4[32 * jj:32 * jj + 1, :],
                            v_sb[:, dc:dc + 1],
                            tts[dc][:, kb * 512:(kb + 1) * 512],
                            start=(dc == 0), stop=(dc == 1),
                            skip_group_check=True,
                            tile_position=(0, 32 * jj),
                        )
                if kp == 1:
                    # out[b, :] = 10 * tanh(strips); junk rows are free
                    t5 = tailp.tile([128, 512], dt.float32, tag="t5")
                    nc.scalar.activation(t5[:], st4[:], AF.Tanh)
                    o5 = tailp.tile([128, 512], dt.float32, tag="o5")
                    nc.vector.tensor_scalar_mul(o5[:], t5[:], C_CLIP)
                    # SWDGE ring (gpsimd): decoupled from the bulk enc queue
                    nc.gpsimd.dma_start(out_p[b:b + 1, :], o5[0:128:32, :])

            pend = None
            NSLAB = N // SLAB_N
            for s in range(NSLAB):                  # 8 slabs of 4 b-rows
                nxt_slab = load_slab(s + 1, pieces=2) if s + 1 < NSLAB else None
                for b_in in range(SLAB_B):
                    b = SLAB_B * s + b_in
                    st4 = psp.tile([128, 512], dt.float32, tag="st")
                    for kp in range(2):             # two 1024-wide n groups
                        tts = []
                        for dc in range(2):
                            psd = pmp.tile([128, 1024], dt.float32, tag="psd")
                            # ec outer / kb inner: consecutive matmuls share
                            # the stationary operand -> fewer weight loads
                            for ec in range(2):
                                for kb in range(2):
                                    nseg = b_in * K + kp * 1024 + kb * 512
                                    nc.tensor.matmul(
                                        psd[:, kb * 512:(kb + 1) * 512],
                                        wref_sb[:, (ec * 2 + dc) * 128:(ec * 2 + dc + 1) * 128],
                                        cur_slab[ec][:, nseg:nseg + 512],
                                        start=(ec == 0), stop=(ec == 1),
                                        skip_group_check=True,
                                    )
                            ttile = tp.tile([128, 1024], dt.bfloat16, tag="tt")
                            nc.scalar.activation(
                                ttile[:], psd[:], AF.Tanh,
                                bias=bias_sb[:, dc * BL + b:dc * BL + b + 1],
                                scale=1.0)
                            tts.append(ttile)
                        if pend is not None:
                            emit_epilogue(*pend)
                        pend = (st4, tts, b, kp)
                cur_slab = nxt_slab
            emit_epilogue(*pend)

    nc.compile()
    return nc


def _prep_inputs(encoder_output, query, Wq, bq, Wref, bref, v):
    bf16 = ml_dtypes.bfloat16
    # (K, B, E) -> (E, B, K), bf16
    enc_bf = np.asarray(encoder_output, np.float32).astype(bf16)
    encT = enc_bf.transpose(2, 1, 0)                   # (E, B, K) view

    def chunk4(w):                                     # (E, D) -> (4*128, 128)
        return np.ascontiguousarray(
            w.reshape(2, 128, 2, 128).transpose(0, 2, 1, 3).reshape(512, 128))

    wrefT = chunk4(np.asarray(Wref, np.float32).T).astype(bf16)
    wqT = chunk4(np.asarray(Wq, np.float32).T)
    cbias = (np.asarray(bref, np.float32) + np.asarray(bq, np.float32)).reshape(E, 1)
    v_col = np.asarray(v, np.float32).astype(bf16).reshape(E, 1)
    queryT = np.ascontiguousarray(np.asarray(query, np.float32).T)  # (E, B)

    in_maps = []
    for c in range(NCORES):
        enc_c = np.ascontiguousarray(encT[:, c * BL:(c + 1) * BL, :]).reshape(E, N)
        in_maps.append({
            "enc_t": enc_c,
            "wref_t": wrefT,
            "wq_t": wqT,
            "query_t": np.ascontiguousarray(queryT[:, c * BL:(c + 1) * BL]),
            "cbias_t": cbias,
            "v_t": v_col,
        })
    return in_maps


def kernel(**inputs):
    global _compiled, last_exec_time_ns, last_results
    from concourse import bass_utils

    if _compiled is None:
        _compiled = _build()
    nc = _compiled

    in_maps = _prep_inputs(**inputs)
    res = bass_utils.run_bass_kernel_spmd(nc, in_maps, core_ids=list(range(NCORES)))
    last_exec_time_ns = res.exec_time_ns
    last_results = res
    out = np.concatenate([r["out"] for r in res.results], axis=0)
    return out


# revision 26
# speedup vs baseline: 1.2340x; 1.2183x over previous
"""Trainium2 Bass kernel for the attention-scoring module:

    q = query @ Wq.T + bq                               # (B, D)
    ref[b,d,k] = sum_e enc[k,b,e] * Wref[d,e] + bref[d]
    u[b,k] = sum_d v[d] * tanh(ref[b,d,k] + q[b,d])
    out = 10 * tanh(u)                                  # (B, K)

Data-parallel over batch: core c owns b in [32c, 32c+32).

Per-core dataflow (all big tensors bf16, f32 accumulation):
  - host pre-transposes enc to (E, b*K+k) so the contraction dim E lands
    on SBUF partitions with dense DMA.
  - main matmuls: psum[d(128), n(512)] += WrefT_chunk.T @ encT_chunk
  - bias (bref+bq+q_raw[b])[d] is per-partition in this layout -> folded
    into the ScalarE tanh activation for free.
  - the v-weighted d-reduction is a second-level matmul with stationary
    v (128,1): strips (1, 512) for the four k-blocks of one b land at
    partitions {0,32,64,96} of one PSUM bank via tile_position col
    groups (bank pre-zeroed, accumulation via start=False).
  - final 10*tanh(u) runs on the whole strip window (junk rows are
    free); the per-b output DMA plucks rows {0,32,64,96} with a
    stepped-partition access pattern.
"""

import os
import sys

import numpy as np

for _p in ("/opt/trn_rl_repo", "/opt/pypackages"):
    if _p not in sys.path:
        sys.path.append(_p)

import ml_dtypes

E = 256
D = 256
K = 2048
B = 256
NCORES = 8
BL = B // NCORES          # 32 batch rows per core
N = BL * K                # 65536 flattened (b, k) per core
SLAB_B = 4                # b-rows per enc DMA slab
SLAB_N = SLAB_B * K       # 8192
C_CLIP = 10.0

_compiled = None
last_exec_time_ns = None
last_results = None


def _build():
    from concourse import bacc, bass, tile

    mybir = bass.mybir
    dt = mybir.dt
    AF = mybir.ActivationFunctionType

    nc = bacc.Bacc("TRN2", target_bir_lowering=False, debug=False,
                   num_devices=NCORES)

    enc_t = nc.declare_dram_parameter("enc_t", [E, N], dt.bfloat16, isOutput=False)
    wref_t = nc.declare_dram_parameter("wref_t", [4 * 128, 128], dt.bfloat16, isOutput=False)
    wq_t = nc.declare_dram_parameter("wq_t", [4 * 128, 128], dt.float32, isOutput=False)
    query_t = nc.declare_dram_parameter("query_t", [E, BL], dt.float32, isOutput=False)
    cbias_t = nc.declare_dram_parameter("cbias_t", [E, 1], dt.float32, isOutput=False)
    v_t = nc.declare_dram_parameter("v_t", [E, 1], dt.bfloat16, isOutput=False)
    out_p = nc.declare_dram_parameter("out", [BL, K], dt.float32, isOutput=True)

    with tile.TileContext(nc) as tc:
        with (
            tc.tile_pool(name="const", bufs=1) as constp,
            tc.tile_pool(name="enc", bufs=3) as encp,
            tc.tile_pool(name="tt", bufs=6) as tp,
            tc.tile_pool(name="tail", bufs=2) as tailp,
            tc.tile_pool(name="psum_m", bufs=3, space="PSUM") as pmp,
            tc.tile_pool(name="psum_s", bufs=2, space="PSUM") as psp,
        ):
            # ---- enc slab loading (slab 0 first, split per b-row so the
            # first matmuls start as early as possible) ----
            def load_slab(s, pieces):
                # ec-interleaved pieces: the first matmuls of the slab wait
                # only on the first piece of each e-chunk
                tiles = [encp.tile([128, SLAB_N], dt.bfloat16, tag=f"enc{ec}",
                                   name=f"enc{ec}_s{s}")
                         for ec in range(2)]
                w = SLAB_N // pieces
                for q in range(pieces):
                    for ec in range(2):
                        nc.sync.dma_start(
                            tiles[ec][:, q * w:(q + 1) * w],
                            enc_t[ec * 128:(ec + 1) * 128,
                                  s * SLAB_N + q * w:s * SLAB_N + (q + 1) * w])
                return tiles

            # ---- constants (weights first so the first matmuls are not
            # stuck behind bulk enc traffic in the DMA queue) ----
            wref_sb = constp.tile([128, 512], dt.bfloat16)   # [:, (ec*2+dc)*128 + d]
            wq_sb = constp.tile([128, 512], dt.float32)
            query_sb = constp.tile([128, 2 * BL], dt.float32)  # [:, ec*32 + b]
            cbias_sb = constp.tile([128, 2], dt.float32)
            v_sb = constp.tile([128, 2], dt.bfloat16)
            bias_sb = constp.tile([128, 2 * BL], dt.float32)   # [:, dc*32 + b]

            for c in range(4):
                nc.sync.dma_start(wq_sb[:, c * 128:(c + 1) * 128],
                                  wq_t[c * 128:(c + 1) * 128, :])
            for ec in range(2):
                nc.sync.dma_start(query_sb[:, ec * BL:(ec + 1) * BL],
                                  query_t[ec * 128:(ec + 1) * 128, :])
            for c in range(4):
                nc.sync.dma_start(wref_sb[:, c * 128:(c + 1) * 128],
                                  wref_t[c * 128:(c + 1) * 128, :])
            for dc in range(2):
                nc.sync.dma_start(cbias_sb[:, dc:dc + 1],
                                  cbias_t[dc * 128:(dc + 1) * 128, :])
                nc.sync.dma_start(v_sb[:, dc:dc + 1],
                                  v_t[dc * 128:(dc + 1) * 128, :])

            cur_slab = load_slab(0, pieces=SLAB_B)

            # ---- q_rawT = (query @ Wq.T).T per d-chunk, + (bref + bq) ----
            for dc in range(2):
                qps = psp.tile([128, BL], dt.float32, tag="st")
                for ec in range(2):
                    nc.tensor.matmul(
                        qps[:],
                        wq_sb[:, (ec * 2 + dc) * 128:(ec * 2 + dc + 1) * 128],
                        query_sb[:, ec * BL:(ec + 1) * BL],
                        start=(ec == 0), stop=(ec == 1),
                    )
                nc.vector.tensor_scalar_add(bias_sb[:, dc * BL:(dc + 1) * BL],
                                            qps[:], cbias_sb[:, dc:dc + 1])

            # ---- main loop, v-matmuls software-pipelined one round behind
            # the main matmuls so they never stall TensorE on ScalarE ----
            def emit_epilogue(st4, tts, b, kp):
                for kb in range(2):
                    jj = kp * 2 + kb
                    for dc in range(2):
                        # start=True clears has_written per element, so the
                        # sibling strips in the same bank are unaffected
                        nc.tensor.matmul(
                            st4[32 * jj:32 * jj + 1, :],
                            v_sb[:, dc:dc + 1],
                            tts[dc][:, kb * 512:(kb + 1) * 512],
                            start=(dc == 0), stop=(dc == 1),
                            skip_group_check=True,
                            tile_position=(0, 32 * jj),
                        )
                if kp == 1:
                    # out[b, :] = 10 * tanh(strips); junk rows are free
                    t5 = tailp.tile([128, 512], dt.float32, tag="t5")
                    nc.scalar.activation(t5[:], st4[:], AF.Tanh)
                    o5 = tailp.tile([128, 512], dt.float32, tag="o5")
                    nc.vector.tensor_scalar_mul(o5[:], t5[:], C_CLIP)
                    # SWDGE ring (gpsimd): decoupled from the bulk enc queue
                    nc.gpsimd.dma_start(out_p[b:b + 1, :], o5[0:128:32, :])

            pend = None
            NSLAB = N // SLAB_N
            for s in range(NSLAB):                  # 8 slabs of 4 b-rows
                nxt_slab = None
                for b_in in range(SLAB_B):
                    if b_in == 1 and s + 1 < NSLAB:
                        # deferred so the prefetch doesn't steal SDMA
                        # bandwidth from this slab's own pieces
                        nxt_slab = load_slab(s + 1, pieces=2)
                    b = SLAB_B * s + b_in
                    st4 = psp.tile([128, 512], dt.float32, tag="st")
                    for kp in range(2):             # two 1024-wide n groups
                        tts = []
                        for dc in range(2):
                            psd = pmp.tile([128, 1024], dt.float32, tag="psd")
                            # ec outer / kb inner: consecutive matmuls share
                            # the stationary operand -> fewer weight loads
                            for ec in range(2):
                                for kb in range(2):
                                    nseg = b_in * K + kp * 1024 + kb * 512
                                    nc.tensor.matmul(
                                        psd[:, kb * 512:(kb + 1) * 512],
                                        wref_sb[:, (ec * 2 + dc) * 128:(ec * 2 + dc + 1) * 128],
                                        cur_slab[ec][:, nseg:nseg + 512],
                                        start=(ec == 0), stop=(ec == 1),
                                        skip_group_check=True,
                                    )
                            ttile = tp.tile([128, 1024], dt.bfloat16, tag="tt")
                            nc.scalar.activation(
                                ttile[:], psd[:], AF.Tanh,
                                bias=bias_sb[:, dc * BL + b:dc * BL + b + 1],
                                scale=1.0)
                            tts.append(ttile)
                        if pend is not None:
                            emit_epilogue(*pend)
                        pend = (st4, tts, b, kp)
                cur_slab = nxt_slab
            emit_epilogue(*pend)

    nc.compile()
    return nc


def _prep_inputs(encoder_output, query, Wq, bq, Wref, bref, v):
    bf16 = ml_dtypes.bfloat16
    # (K, B, E) -> (E, B, K), bf16
    enc_bf = np.asarray(encoder_output, np.float32).astype(bf16)
    encT = enc_bf.transpose(2, 1, 0)                   # (E, B, K) view

    def chunk4(w):                                     # (E, D) -> (4*128, 128)
        return np.ascontiguousarray(
            w.reshape(2, 128, 2, 128).transpose(0, 2, 1, 3).reshape(512, 128))

    wrefT = chunk4(np.asarray(Wref, np.float32).T).astype(bf16)
    wqT = chunk4(np.asarray(Wq, np.float32).T)
    cbias = (np.asarray(bref, np.float32) + np.asarray(bq, np.float32)).reshape(E, 1)
    v_col = np.asarray(v, np.float32).astype(bf16).reshape(E, 1)
    queryT = np.ascontiguousarray(np.asarray(query, np.float32).T)  # (E, B)

    in_maps = []
    for c in range(NCORES):
        enc_c = np.ascontiguousarray(encT[:, c * BL:(c + 1) * BL, :]).reshape(E, N)
        in_maps.append({
            "enc_t": enc_c,
            "wref_t": wrefT,
            "wq_t": wqT,
            "query_t": np.ascontiguousarray(queryT[:, c * BL:(c + 1) * BL]),
            "cbias_t": cbias,
            "v_t": v_col,
        })
    return in_maps


def kernel(**inputs):
    global _compiled, last_exec_time_ns, last_results
    from concourse import bass_utils

    if _compiled is None:
        _compiled = _build()
    nc = _compiled

    in_maps = _prep_inputs(**inputs)
    res = bass_utils.run_bass_kernel_spmd(nc, in_maps, core_ids=list(range(NCORES)))
    last_exec_time_ns = res.exec_time_ns
    last_results = res
    out = np.concatenate([r["out"] for r in res.results], axis=0)
    return out


# revision 34
# speedup vs baseline: 1.2352x; 1.0009x over previous
"""Trainium2 Bass kernel for the attention-scoring module:

    q = query @ Wq.T + bq                               # (B, D)
    ref[b,d,k] = sum_e enc[k,b,e] * Wref[d,e] + bref[d]
    u[b,k] = sum_d v[d] * tanh(ref[b,d,k] + q[b,d])
    out = 10 * tanh(u)                                  # (B, K)

Data-parallel over batch: core c owns b in [32c, 32c+32).

Per-core dataflow (all big tensors bf16, f32 accumulation):
  - host pre-transposes enc to (E, b*K+k) so the contraction dim E lands
    on SBUF partitions with dense DMA.
  - main matmuls: psum[d(128), n(512)] += WrefT_chunk.T @ encT_chunk
  - bias (bref+bq+q_raw[b])[d] is per-partition in this layout -> folded
    into the ScalarE tanh activation for free.
  - the v-weighted d-reduction is a second-level matmul with stationary
    v (128,1): strips (1, 512) for the four k-blocks of one b land at
    partitions {0,32,64,96} of one PSUM bank via tile_position col
    groups (bank pre-zeroed, accumulation via start=False).
  - final 10*tanh(u) runs on the whole strip window (junk rows are
    free); the per-b output DMA plucks rows {0,32,64,96} with a
    stepped-partition access pattern.
"""

import os
import sys

import numpy as np

os.environ.setdefault("JAX_COMPILATION_CACHE_DIR", "/tmp/jaxcache")

for _p in ("/opt/trn_rl_repo", "/opt/pypackages"):
    if _p not in sys.path:
        sys.path.append(_p)

import ml_dtypes

E = 256
D = 256
K = 2048
B = 256
NCORES = 8
BL = B // NCORES          # 32 batch rows per core
N = BL * K                # 65536 flattened (b, k) per core
SLAB_B = 4                # b-rows per enc DMA slab
SLAB_N = SLAB_B * K       # 8192
C_CLIP = 10.0

_compiled = None
last_exec_time_ns = None
last_results = None


def _build():
    from concourse import bacc, bass, tile

    mybir = bass.mybir
    dt = mybir.dt
    AF = mybir.ActivationFunctionType

    nc = bacc.Bacc("TRN2", target_bir_lowering=False, debug=False,
                   num_devices=NCORES)

    enc_t = nc.declare_dram_parameter("enc_t", [E, N], dt.bfloat16, isOutput=False)
    wref_t = nc.declare_dram_parameter("wref_t", [4 * 128, 128], dt.bfloat16, isOutput=False)
    wq_t = nc.declare_dram_parameter("wq_t", [4 * 128, 128], dt.float32, isOutput=False)
    query_t = nc.declare_dram_parameter("query_t", [E, BL], dt.float32, isOutput=False)
    cbias_t = nc.declare_dram_parameter("cbias_t", [E, 1], dt.float32, isOutput=False)
    v_t = nc.declare_dram_parameter("v_t", [E, 1], dt.bfloat16, isOutput=False)
    out_p = nc.declare_dram_parameter("out", [BL, K], dt.float32, isOutput=True)

    with tile.TileContext(nc) as tc:
        with (
            tc.tile_pool(name="const", bufs=1) as constp,
            tc.tile_pool(name="enc", bufs=3) as encp,
            tc.tile_pool(name="tt", bufs=6) as tp,
            tc.tile_pool(name="tail", bufs=2) as tailp,
            tc.tile_pool(name="psum_m", bufs=3, space="PSUM") as pmp,
            tc.tile_pool(name="psum_s", bufs=2, space="PSUM") as psp,
        ):
            # ---- enc slab loading (slab 0 first, split per b-row so the
            # first matmuls start as early as possible) ----
            def alloc_slab(s):
                return [encp.tile([128, SLAB_N], dt.bfloat16, tag=f"enc{ec}",
                                  name=f"enc{ec}_s{s}")
                        for ec in range(2)]

            def emit_pieces(tiles, s, q0, q1, pieces=SLAB_B, dep=None):
                # ec-interleaved pieces q0..q1-1 of slab s; `dep` gates the
                # DMA issue so queued prefetches don't fair-share SDMA
                # bandwidth away from pieces that are needed right now
                w = SLAB_N // pieces
                for q in range(q0, q1):
                    for ec in range(2):
                        ins = nc.sync.dma_start(
                            tiles[ec][:, q * w:(q + 1) * w],
                            enc_t[ec * 128:(ec + 1) * 128,
                                  s * SLAB_N + q * w:s * SLAB_N + (q + 1) * w])
                        if dep is not None:
                            tile.add_dep_helper(ins.ins, dep.ins,
                                                reason="defer enc prefetch")

            def load_slab(s, pieces, dep=None):
                tiles = alloc_slab(s)
                emit_pieces(tiles, s, 0, pieces, pieces, dep=dep)
                return tiles

            # ---- constants (weights first so the first matmuls are not
            # stuck behind bulk enc traffic in the DMA queue) ----
            wref_sb = constp.tile([128, 512], dt.bfloat16)   # [:, (ec*2+dc)*128 + d]
            wq_sb = constp.tile([128, 512], dt.float32)
            query_sb = constp.tile([128, 2 * BL], dt.float32)  # [:, ec*32 + b]
            cbias_sb = constp.tile([128, 2], dt.float32)
            v_sb = constp.tile([128, 2], dt.bfloat16)
            bias_sb = constp.tile([128, 2 * BL], dt.float32)   # [:, dc*32 + b]

            for c in range(4):
                nc.sync.dma_start(wq_sb[:, c * 128:(c + 1) * 128],
                                  wq_t[c * 128:(c + 1) * 128, :])
            for ec in range(2):
                nc.sync.dma_start(query_sb[:, ec * BL:(ec + 1) * BL],
                                  query_t[ec * 128:(ec + 1) * 128, :])
            for c in range(4):
                nc.sync.dma_start(wref_sb[:, c * 128:(c + 1) * 128],
                                  wref_t[c * 128:(c + 1) * 128, :])
            for dc in range(2):
                nc.sync.dma_start(cbias_sb[:, dc:dc + 1],
                                  cbias_t[dc * 128:(dc + 1) * 128, :])
                nc.sync.dma_start(v_sb[:, dc:dc + 1],
                                  v_t[dc * 128:(dc + 1) * 128, :])

            # slab 0 staged: only b0's piece upfront, the rest fed into the
            # pipeline so the first matmuls aren't starved by SDMA
            # fair-sharing across all queued pieces
            slab0 = alloc_slab(0)
            emit_pieces(slab0, 0, 0, 1)

            # ---- q_rawT = (query @ Wq.T).T per d-chunk, + (bref + bq) ----
            for dc in range(2):
                qps = psp.tile([128, BL], dt.float32, tag="st")
                for ec in range(2):
                    nc.tensor.matmul(
                        qps[:],
                        wq_sb[:, (ec * 2 + dc) * 128:(ec * 2 + dc + 1) * 128],
                        query_sb[:, ec * BL:(ec + 1) * BL],
                        start=(ec == 0), stop=(ec == 1),
                    )
                nc.vector.tensor_scalar_add(bias_sb[:, dc * BL:(dc + 1) * BL],
                                            qps[:], cbias_sb[:, dc:dc + 1])

            # ---- main loop, v-matmuls software-pipelined one round behind
            # the main matmuls so they never stall TensorE on ScalarE ----
            def emit_epilogue(st4, tts, b, kp):
                for kb in range(2):
                    jj = kp * 2 + kb
                    for dc in range(2):
                        # start=True clears has_written per element, so the
                        # sibling strips in the same bank are unaffected
                        nc.tensor.matmul(
                            st4[32 * jj:32 * jj + 1, :],
                            v_sb[:, dc:dc + 1],
                            tts[dc][:, kb * 512:(kb + 1) * 512],
                            start=(dc == 0), stop=(dc == 1),
                            skip_group_check=True,
                            tile_position=(0, 32 * jj),
                        )
                if kp == 1:
                    # out[b, :] = 10 * tanh(strips); junk rows are free
                    t5 = tailp.tile([128, 512], dt.float32, tag="t5")
                    nc.scalar.activation(t5[:], st4[:], AF.Tanh)
                    o5 = tailp.tile([128, 512], dt.float32, tag="o5")
                    nc.vector.tensor_scalar_mul(o5[:], t5[:], C_CLIP)
                    # SWDGE ring (gpsimd): decoupled from the bulk enc queue
                    nc.gpsimd.dma_start(out_p[b:b + 1, :], o5[0:128:32, :])

            pend = None
            NSLAB = N // SLAB_N
            cur_slab = slab0
            prev_mm = None      # first main matmul of the previous round
            for s in range(NSLAB):                  # 8 slabs of 4 b-rows
                nxt_slab = None
                for b_in in range(SLAB_B):
                    b = SLAB_B * s + b_in
                    st4 = psp.tile([128, 512], dt.float32, tag="st")
                    for kp in range(2):             # two 1024-wide n groups
                        if s == 0 and (b_in, kp) == (0, 1):
                            emit_pieces(cur_slab, 0, 1, 2, dep=prev_mm)
                        if s == 0 and (b_in, kp) == (1, 0):
                            emit_pieces(cur_slab, 0, 2, 4, dep=prev_mm)
                        pf_at = 2 if s == 0 else 1
                        if (b_in, kp) == (pf_at, 0) and s + 1 < NSLAB:
                            nxt_slab = load_slab(s + 1, pieces=2, dep=prev_mm)
                        first_mm = None
                        tts = []
                        for dc in range(2):
                            psd = pmp.tile([128, 1024], dt.float32, tag="psd")
                            # ec outer / kb inner: consecutive matmuls share
                            # the stationary operand -> fewer weight loads
                            for ec in range(2):
                                for kb in range(2):
                                    nseg = b_in * K + kp * 1024 + kb * 512
                                    ins = nc.tensor.matmul(
                                        psd[:, kb * 512:(kb + 1) * 512],
                                        wref_sb[:, (ec * 2 + dc) * 128:(ec * 2 + dc + 1) * 128],
                                        cur_slab[ec][:, nseg:nseg + 512],
                                        start=(ec == 0), stop=(ec == 1),
                                        skip_group_check=True,
                                    )
                                    if first_mm is None:
                                        first_mm = ins
                            ttile = tp.tile([128, 1024], dt.bfloat16, tag="tt")
                            nc.scalar.activation(
                                ttile[:], psd[:], AF.Tanh,
                                bias=bias_sb[:, dc * BL + b:dc * BL + b + 1],
                                scale=1.0)
                            tts.append(ttile)
                        if pend is not None:
                            emit_epilogue(*pend)
                        pend = (st4, tts, b, kp)
                        prev_mm = first_mm
                cur_slab = nxt_slab
            emit_epilogue(*pend)

    nc.compile()
    return nc


def _prep_inputs(encoder_output, query, Wq, bq, Wref, bref, v):
    bf16 = ml_dtypes.bfloat16
    # (K, B, E) -> (E, B, K), bf16
    enc_bf = np.asarray(encoder_output, np.float32).astype(bf16)
    encT = enc_bf.transpose(2, 1, 0)                   # (E, B, K) view

    def chunk4(w):                                     # (E, D) -> (4*128, 128)
        return np.ascontiguousarray(
            w.reshape(2, 128, 2, 128).transpose(0, 2, 1, 3).reshape(512, 128))

    wrefT = chunk4(np.asarray(Wref, np.float32).T).astype(bf16)
    wqT = chunk4(np.asarray(Wq, np.float32).T)
    cbias = (np.asarray(bref, np.float32) + np.asarray(bq, np.float32)).reshape(E, 1)
    v_col = np.asarray(v, np.float32).astype(bf16).reshape(E, 1)
    queryT = np.ascontiguousarray(np.asarray(query, np.float32).T)  # (E, B)

    in_maps = []
    for c in range(NCORES):
        enc_c = np.ascontiguousarray(encT[:, c * BL:(c + 1) * BL, :]).reshape(E, N)
        in_maps.append({
            "enc_t": enc_c,
            "wref_t": wrefT,
            "wq_t": wqT,
            "query_t": np.ascontiguousarray(queryT[:, c * BL:(c + 1) * BL]),
            "cbias_t": cbias,
            "v_t": v_col,
        })
    return in_maps


def kernel(**inputs):
    global _compiled, last_exec_time_ns, last_results
    from concourse import bass_utils

    if _compiled is None:
        _compiled = _build()
    nc = _compiled

    in_maps = _prep_inputs(**inputs)
    res = bass_utils.run_bass_kernel_spmd(nc, in_maps, core_ids=list(range(NCORES)))
    last_exec_time_ns = res.exec_time_ns
    last_results = res
    out = np.concatenate([r["out"] for r in res.results], axis=0)
    return out


# revision 37
# speedup vs baseline: 1.2761x; 1.0331x over previous
"""Trainium2 Bass kernel for the attention-scoring module:

    q = query @ Wq.T + bq                               # (B, D)
    ref[b,d,k] = sum_e enc[k,b,e] * Wref[d,e] + bref[d]
    u[b,k] = sum_d v[d] * tanh(ref[b,d,k] + q[b,d])
    out = 10 * tanh(u)                                  # (B, K)

Data-parallel over batch: core c owns b in [32c, 32c+32).

Per-core dataflow (all big tensors bf16, f32 accumulation):
  - host pre-transposes enc to (E, b*K+k) so the contraction dim E lands
    on SBUF partitions with dense DMA.
  - main matmuls: psum[d(128), n(512)] += WrefT_chunk.T @ encT_chunk
  - bias (bref+bq+q_raw[b])[d] is per-partition in this layout -> folded
    into the ScalarE tanh activation for free.
  - the v-weighted d-reduction is a second-level matmul with stationary
    v (128,1): strips (1, 512) for the four k-blocks of one b land at
    partitions {0,32,64,96} of one PSUM bank via tile_position col
    groups (bank pre-zeroed, accumulation via start=False).
  - final 10*tanh(u) runs on the whole strip window (junk rows are
    free); the per-b output DMA plucks rows {0,32,64,96} with a
    stepped-partition access pattern.
"""

import os
import sys

import numpy as np

os.environ.setdefault("JAX_COMPILATION_CACHE_DIR", "/tmp/jaxcache")

for _p in ("/opt/trn_rl_repo", "/opt/pypackages"):
    if _p not in sys.path:
        sys.path.append(_p)

import ml_dtypes

E = 256
D = 256
K = 2048
B = 256
NCORES = 8
BL = B // NCORES          # 32 batch rows per core
N = BL * K                # 65536 flattened (b, k) per core
SLAB_B = 4                # b-rows per enc DMA slab
SLAB_N = SLAB_B * K       # 8192
C_CLIP = 10.0

_compiled = None
last_exec_time_ns = None
last_results = None


def _build():
    from concourse import bacc, bass, tile

    mybir = bass.mybir
    dt = mybir.dt
    AF = mybir.ActivationFunctionType

    nc = bacc.Bacc("TRN2", target_bir_lowering=False, debug=False,
                   num_devices=NCORES)

    enc_t = nc.declare_dram_parameter("enc_t", [E, N], dt.bfloat16, isOutput=False)
    # all small constants pre-packed host-side into two tensors so startup
    # is 2 DMAs, not 14 (each dma_start costs ~0.5us of queue issue latency)
    cf32_t = nc.declare_dram_parameter("cf32", [128, 578], dt.float32, isOutput=False)
    cbf16_t = nc.declare_dram_parameter("cbf16", [128, 514], dt.bfloat16, isOutput=False)
    out_p = nc.declare_dram_parameter("out", [BL, K], dt.float32, isOutput=True)

    with tile.TileContext(nc) as tc:
        with (
            tc.tile_pool(name="const", bufs=1) as constp,
            tc.tile_pool(name="enc", bufs=3) as encp,
            tc.tile_pool(name="tt", bufs=6) as tp,
            tc.tile_pool(name="tail", bufs=2) as tailp,
            tc.tile_pool(name="psum_m", bufs=3, space="PSUM") as pmp,
            tc.tile_pool(name="psum_s", bufs=2, space="PSUM") as psp,
        ):
            # ---- enc slab loading (slab 0 first, split per b-row so the
            # first matmuls start as early as possible) ----
            def alloc_slab(s):
                return [encp.tile([128, SLAB_N], dt.bfloat16, tag=f"enc{ec}",
                                  name=f"enc{ec}_s{s}")
                        for ec in range(2)]

            def emit_pieces(tiles, s, q0, q1, pieces=SLAB_B, dep=None):
                # ec-interleaved pieces q0..q1-1 of slab s; `dep` gates the
                # DMA issue so queued prefetches don't fair-share SDMA
                # bandwidth away from pieces that are needed right now
                w = SLAB_N // pieces
                for q in range(q0, q1):
                    for ec in range(2):
                        ins = nc.sync.dma_start(
                            tiles[ec][:, q * w:(q + 1) * w],
                            enc_t[ec * 128:(ec + 1) * 128,
                                  s * SLAB_N + q * w:s * SLAB_N + (q + 1) * w])
                        if dep is not None:
                            tile.add_dep_helper(ins.ins, dep.ins,
                                                reason="defer enc prefetch")

            def load_slab(s, pieces, dep=None):
                tiles = alloc_slab(s)
                emit_pieces(tiles, s, 0, pieces, pieces, dep=dep)
                return tiles

            # ---- constants: two packed DMAs ----
            cf32_sb = constp.tile([128, 578], dt.float32)
            cbf16_sb = constp.tile([128, 514], dt.bfloat16)
            bias_sb = constp.tile([128, 2 * BL], dt.float32)   # [:, dc*32 + b]
            nc.sync.dma_start(cf32_sb[:], cf32_t[:])
            nc.sync.dma_start(cbf16_sb[:], cbf16_t[:])

            wq_sb = cf32_sb[:, 0:512]        # [:, (ec*2+dc)*128 + d]
            query_sb = cf32_sb[:, 512:576]   # [:, ec*32 + b]
            cbias_sb = cf32_sb[:, 576:578]
            wref_sb = cbf16_sb[:, 0:512]     # [:, (ec*2+dc)*128 + d]
            v_sb = cbf16_sb[:, 512:514]

            # slab 0 staged: only b0's piece upfront, the rest fed into the
            # pipeline so the first matmuls aren't starved by SDMA
            # fair-sharing across all queued pieces
            slab0 = alloc_slab(0)
            emit_pieces(slab0, 0, 0, 1)

            # ---- q_rawT = (query @ Wq.T).T per d-chunk, + (bref + bq) ----
            for dc in range(2):
                qps = psp.tile([128, BL], dt.float32, tag="st")
                for ec in range(2):
                    nc.tensor.matmul(
                        qps[:],
                        wq_sb[:, (ec * 2 + dc) * 128:(ec * 2 + dc + 1) * 128],
                        query_sb[:, ec * BL:(ec + 1) * BL],
                        start=(ec == 0), stop=(ec == 1),
                    )
                nc.vector.tensor_scalar_add(bias_sb[:, dc * BL:(dc + 1) * BL],
                                            qps[:], cbias_sb[:, dc:dc + 1])

            # ---- main loop, v-matmuls software-pipelined one round behind
            # the main matmuls so they never stall TensorE on ScalarE ----
            def emit_epilogue(st4, tts, b, kp):
                for kb in range(2):
                    jj = kp * 2 + kb
                    for dc in range(2):
                        # start=True clears has_written per element, so the
                        # sibling strips in the same bank are unaffected
                        nc.tensor.matmul(
                            st4[32 * jj:32 * jj + 1, :],
                            v_sb[:, dc:dc + 1],
                            tts[dc][:, kb * 512:(kb + 1) * 512],
                            start=(dc == 0), stop=(dc == 1),
                            skip_group_check=True,
                            tile_position=(0, 32 * jj),
                        )
                if kp == 1:
                    # out[b, :] = 10 * tanh(strips); junk rows are free
                    t5 = tailp.tile([128, 512], dt.float32, tag="t5")
                    nc.scalar.activation(t5[:], st4[:], AF.Tanh)
                    o5 = tailp.tile([128, 512], dt.float32, tag="o5")
                    nc.vector.tensor_scalar_mul(o5[:], t5[:], C_CLIP)
                    # SWDGE ring (gpsimd): decoupled from the bulk enc queue
                    nc.gpsimd.dma_start(out_p[b:b + 1, :], o5[0:128:32, :])

            pend = None
            NSLAB = N // SLAB_N
            cur_slab = slab0
            prev_mm = None      # first main matmul of the previous round
            for s in range(NSLAB):                  # 8 slabs of 4 b-rows
                nxt_slab = None
                for b_in in range(SLAB_B):
                    b = SLAB_B * s + b_in
                    st4 = psp.tile([128, 512], dt.float32, tag="st")
                    for kp in range(2):             # two 1024-wide n groups
                        if s == 0 and (b_in, kp) == (0, 1):
                            emit_pieces(cur_slab, 0, 1, 2, dep=prev_mm)
                        if s == 0 and (b_in, kp) == (1, 0):
                            emit_pieces(cur_slab, 0, 2, 4, dep=prev_mm)
                        pf_at = 2 if s == 0 else 1
                        if (b_in, kp) == (pf_at, 0) and s + 1 < NSLAB:
                            nxt_slab = load_slab(s + 1, pieces=2, dep=prev_mm)
                        first_mm = None
                        tts = []
                        for dc in range(2):
                            psd = pmp.tile([128, 1024], dt.float32, tag="psd")
                            # ec outer / kb inner: consecutive matmuls share
                            # the stationary operand -> fewer weight loads
                            for ec in range(2):
                                for kb in range(2):
                                    nseg = b_in * K + kp * 1024 + kb * 512
                                    ins = nc.tensor.matmul(
                                        psd[:, kb * 512:(kb + 1) * 512],
                                        wref_sb[:, (ec * 2 + dc) * 128:(ec * 2 + dc + 1) * 128],
                                        cur_slab[ec][:, nseg:nseg + 512],
                                        start=(ec == 0), stop=(ec == 1),
                                        skip_group_check=True,
                                    )
                                    if first_mm is None:
                                        first_mm = ins
                            ttile = tp.tile([128, 1024], dt.bfloat16, tag="tt")
                            nc.scalar.activation(
                                ttile[:], psd[:], AF.Tanh,
                                bias=bias_sb[:, dc * BL + b:dc * BL + b + 1],
                                scale=1.0)
                            tts.append(ttile)
                        if pend is not None:
                            emit_epilogue(*pend)
                        pend = (st4, tts, b, kp)
                        prev_mm = first_mm
                cur_slab = nxt_slab
            emit_epilogue(*pend)

    nc.compile()
    return nc


def _prep_inputs(encoder_output, query, Wq, bq, Wref, bref, v):
    bf16 = ml_dtypes.bfloat16
    # (K, B, E) -> (E, B, K), bf16
    enc_bf = np.asarray(encoder_output, np.float32).astype(bf16)
    encT = enc_bf.transpose(2, 1, 0)                   # (E, B, K) view

    def chunk4(w):                                     # (E, D) -> (4*128, 128)
        return np.ascontiguousarray(
            w.reshape(2, 128, 2, 128).transpose(0, 2, 1, 3).reshape(512, 128))

    def pack(w4):                                      # (4*128, X) -> (128, 4*X)
        x = w4.shape[1]
        return w4.reshape(4, 128, x).transpose(1, 0, 2).reshape(128, 4 * x)

    wref_p = pack(chunk4(np.asarray(Wref, np.float32).T))          # (128, 512)
    wq_p = pack(chunk4(np.asarray(Wq, np.float32).T))              # (128, 512)
    cbias = (np.asarray(bref, np.float32) + np.asarray(bq, np.float32))
    cbias_p = cbias.reshape(2, 128).T                               # (128, 2)
    v_p = np.asarray(v, np.float32).reshape(2, 128).T               # (128, 2)
    queryT = np.ascontiguousarray(np.asarray(query, np.float32).T)  # (E, B)

    cbf16 = np.concatenate([wref_p, v_p], axis=1).astype(bf16)      # (128, 514)

    in_maps = []
    for c in range(NCORES):
        enc_c = np.ascontiguousarray(encT[:, c * BL:(c + 1) * BL, :]).reshape(E, N)
        q_c = queryT[:, c * BL:(c + 1) * BL]                        # (256, 32)
        q_p = q_c.reshape(2, 128, BL).transpose(1, 0, 2).reshape(128, 2 * BL)
        cf32 = np.ascontiguousarray(np.concatenate(
            [wq_p, q_p, cbias_p], axis=1), dtype=np.float32)        # (128, 578)
        in_maps.append({
            "enc_t": enc_c,
            "cf32": cf32,
            "cbf16": cbf16,
        })
    return in_maps


def kernel(**inputs):
    global _compiled, last_exec_time_ns, last_results
    from concourse import bass_utils

    if _compiled is None:
        _compiled = _build()
    nc = _compiled

    in_maps = _prep_inputs(**inputs)
    res = bass_utils.run_bass_kernel_spmd(nc, in_maps, core_ids=list(range(NCORES)))
    last_exec_time_ns = res.exec_time_ns
    last_results = res
    out = np.concatenate([r["out"] for r in res.results], axis=0)
    return out


# revision 50
# speedup vs baseline: 1.3355x; 1.0465x over previous
"""Trainium2 Bass kernel for the attention-scoring module:

    q = query @ Wq.T + bq                               # (B, D)
    ref[b,d,k] = sum_e enc[k,b,e] * Wref[d,e] + bref[d]
    u[b,k] = sum_d v[d] * tanh(ref[b,d,k] + q[b,d])
    out = 10 * tanh(u)                                  # (B, K)

Data-parallel over batch: core c owns b in [32c, 32c+32).

Per-core dataflow (all big tensors bf16, f32 accumulation):
  - host pre-transposes enc to (E, b*K+k) so the contraction dim E lands
    on SBUF partitions with dense DMA.
  - main matmuls: psum[d(128), n(512)] += WrefT_chunk.T @ encT_chunk
  - bias (bref+bq+q_raw[b])[d] is per-partition in this layout -> folded
    into the ScalarE tanh activation for free.
  - the v-weighted d-reduction is a second-level matmul with stationary
    v (128,1): strips (1, 512) for the four k-blocks of one b land at
    partitions {0,32,64,96} of one PSUM bank via tile_position col
    groups (bank pre-zeroed, accumulation via start=False).
  - final 10*tanh(u) runs on the whole strip window (junk rows are
    free); the per-b output DMA plucks rows {0,32,64,96} with a
    stepped-partition access pattern.
"""

import os
import sys

import numpy as np

os.environ.setdefault("JAX_COMPILATION_CACHE_DIR", "/tmp/jaxcache")

for _p in ("/opt/trn_rl_repo", "/opt/pypackages"):
    if _p not in sys.path:
        sys.path.append(_p)

import ml_dtypes

E = 256
D = 256
K = 2048
B = 256
NCORES = 8
BL = B // NCORES          # 32 batch rows per core
N = BL * K                # 65536 flattened (b, k) per core
SLAB_B = 4                # b-rows per enc DMA slab
SLAB_N = SLAB_B * K       # 8192
C_CLIP = 10.0

_compiled = None
last_exec_time_ns = None
last_results = None


def _build():
    from concourse import bacc, bass, tile

    mybir = bass.mybir
    dt = mybir.dt
    AF = mybir.ActivationFunctionType

    nc = bacc.Bacc("TRN2", target_bir_lowering=False, debug=False,
                   num_devices=NCORES)

    enc_t = nc.declare_dram_parameter("enc_t", [E, N], dt.bfloat16, isOutput=False)
    # all small constants pre-packed host-side into two tensors so startup
    # is 2 DMAs, not 14 (each dma_start costs ~0.5us of queue issue latency)
    cf32_t = nc.declare_dram_parameter("cf32", [128, 580], dt.float32, isOutput=False)
    cbf16_t = nc.declare_dram_parameter("cbf16", [128, 515], dt.bfloat16, isOutput=False)
    out_p = nc.declare_dram_parameter("out", [128, 512], dt.float32, isOutput=True)

    with tile.TileContext(nc) as tc:
        with (
            tc.tile_pool(name="const", bufs=1) as constp,
            tc.tile_pool(name="enc", bufs=3) as encp,
            tc.tile_pool(name="tt", bufs=6) as tp,
            tc.tile_pool(name="tail", bufs=2) as tailp,
            tc.tile_pool(name="psum_m", bufs=3, space="PSUM") as pmp,
            tc.tile_pool(name="psum_s", bufs=2, space="PSUM") as psp,
        ):
            # ---- enc slab loading (slab 0 first, split per b-row so the
            # first matmuls start as early as possible) ----
            def alloc_slab(s):
                return [encp.tile([128, SLAB_N], dt.bfloat16, tag=f"enc{ec}",
                                  name=f"enc{ec}_s{s}")
                        for ec in range(2)]

            def emit_pieces(tiles, s, q0, q1, pieces=SLAB_B, dep=None):
                # ec-interleaved pieces q0..q1-1 of slab s; `dep` gates the
                # DMA issue so queued prefetches don't fair-share SDMA
                # bandwidth away from pieces that are needed right now
                w = SLAB_N // pieces
                for q in range(q0, q1):
                    for ec in range(2):
                        ins = nc.sync.dma_start(
                            tiles[ec][:, q * w:(q + 1) * w],
                            enc_t[ec * 128:(ec + 1) * 128,
                                  s * SLAB_N + q * w:s * SLAB_N + (q + 1) * w])
                        if dep is not None:
                            tile.add_dep_helper(ins.ins, dep.ins,
                                                reason="defer enc prefetch")

            def load_slab(s, pieces, dep=None):
                tiles = alloc_slab(s)
                emit_pieces(tiles, s, 0, pieces, pieces, dep=dep)
                return tiles

            # ---- constants: two packed DMAs ----
            cf32_sb = constp.tile([128, 580], dt.float32)
            cbf16_sb = constp.tile([128, 515], dt.bfloat16)
            bias_sb = constp.tile([128, 2 * BL], dt.float32)   # [:, dc*32 + b]
            u_sb = constp.tile([128, 512], dt.float32)         # [b*4+jj, kk]
            nc.sync.dma_start(cf32_sb[:], cf32_t[:])
            nc.sync.dma_start(cbf16_sb[:], cbf16_t[:])

            wq_sb = cf32_sb[:, 0:512]        # [:, (ec*2+dc)*128 + d]
            query_sb = cf32_sb[:, 512:576]   # [:, ec*32 + b]
            cbias_sb = cf32_sb[:, 576:578]
            v32_sb = cf32_sb[:, 578:580]     # f32 v for DVE per-partition mults
            wref_sb = cbf16_sb[:, 0:512]     # [:, (ec*2+dc)*128 + d]
            ones_sb = cbf16_sb[:, 514:515]

            # slab 0 staged: only b0's piece upfront, the rest fed into the
            # pipeline so the first matmuls aren't starved by SDMA
            # fair-sharing across all queued pieces
            slab0 = alloc_slab(0)
            emit_pieces(slab0, 0, 0, 1)

            # ---- q_rawT = (query @ Wq.T).T per d-chunk, + (bref + bq) ----
            for dc in range(2):
                qps = psp.tile([128, BL], dt.float32, tag="st")
                for ec in range(2):
                    nc.tensor.matmul(
                        qps[:],
                        wq_sb[:, (ec * 2 + dc) * 128:(ec * 2 + dc + 1) * 128],
                        query_sb[:, ec * BL:(ec + 1) * BL],
                        start=(ec == 0), stop=(ec == 1),
                    )
                nc.vector.tensor_scalar_add(bias_sb[:, dc * BL:(dc + 1) * BL],
                                            qps[:], cbias_sb[:, dc:dc + 1])

            # ---- main loop, v-matmuls software-pipelined one round behind
            # the main matmuls so they never stall TensorE on ScalarE ----
            def emit_epilogue(st4, tts, b, kp):
                # pre-combine the d-chunks on the (idle) VectorE so each
                # strip costs TensorE one ones-matmul instead of two v-mms
                w0 = tp.tile([128, 1024], dt.bfloat16, tag="w0")
                nc.vector.tensor_scalar_mul(w0[:], tts[0][:], v32_sb[:, 0:1])
                w1 = tp.tile([128, 1024], dt.bfloat16, tag="w1")
                nc.vector.tensor_scalar_mul(w1[:], tts[1][:], v32_sb[:, 1:2])
                w = tp.tile([128, 1024], dt.bfloat16, tag="w")
                nc.vector.tensor_add(w[:], w0[:], w1[:])
                for kb in range(2):
                    jj = kp * 2 + kb
                    # start=True clears has_written per element, so the
                    # sibling strips in the same bank are unaffected
                    nc.tensor.matmul(
                        st4[32 * jj:32 * jj + 1, :],
                        ones_sb,
                        w[:, kb * 512:(kb + 1) * 512],
                        start=True, stop=True,
                        skip_group_check=True,
                        tile_position=(0, 32 * jj),
                    )
                if kp == 1:
                    # PSUM egress on DVE; a partition-strided SBUF->SBUF DMA
                    # compacts the 4 live rows into the dense u accumulator
                    sp = tailp.tile([128, 512], dt.float32, tag="sp")
                    nc.vector.tensor_copy(sp[:], st4[:])
                    nc.gpsimd.dma_start(u_sb[4 * b:4 * b + 4, :],
                                        sp[0:128:32, :])

            pend = None
            NSLAB = N // SLAB_N
            cur_slab = slab0
            prev_mm = None      # first main matmul of the previous round
            for s in range(NSLAB):                  # 8 slabs of 4 b-rows
                nxt_slab = None
                for b_in in range(SLAB_B):
                    b = SLAB_B * s + b_in
                    st4 = psp.tile([128, 512], dt.float32, tag="st")
                    for kp in range(2):             # two 1024-wide n groups
                        if s == 0 and (b_in, kp) == (0, 1):
                            emit_pieces(cur_slab, 0, 1, 2, dep=prev_mm)
                        if s == 0 and (b_in, kp) == (1, 0):
                            emit_pieces(cur_slab, 0, 2, 4, dep=prev_mm)
                        pf_at = 2 if s == 0 else 1
                        if (b_in, kp) == (pf_at, 0) and s + 1 < NSLAB:
                            nxt_slab = load_slab(s + 1, pieces=2, dep=prev_mm)
                        first_mm = None
                        tts = []
                        for dc in range(2):
                            psd = pmp.tile([128, 1024], dt.float32, tag="psd")
                            # ec outer / kb inner: consecutive matmuls share
                            # the stationary operand -> fewer weight loads
                            for ec in range(2):
                                for kb in range(2):
                                    nseg = b_in * K + kp * 1024 + kb * 512
                                    ins = nc.tensor.matmul(
                                        psd[:, kb * 512:(kb + 1) * 512],
                                        wref_sb[:, (ec * 2 + dc) * 128:(ec * 2 + dc + 1) * 128],
                                        cur_slab[ec][:, nseg:nseg + 512],
                                        start=(ec == 0), stop=(ec == 1),
                                        skip_group_check=True,
                                    )
                                    if first_mm is None:
                                        first_mm = ins
                            ttile = tp.tile([128, 1024], dt.bfloat16, tag="tt")
                            nc.scalar.activation(
                                ttile[:], psd[:], AF.Tanh,
                                bias=bias_sb[:, dc * BL + b:dc * BL + b + 1],
                                scale=1.0)
                            tts.append(ttile)
                        if pend is not None:
                            emit_epilogue(*pend)
                        pend = (st4, tts, b, kp)
                        prev_mm = first_mm
                cur_slab = nxt_slab
            emit_epilogue(*pend)

            # ---- final: out = 10 * tanh(u), one dense pass ----
            t6 = tailp.tile([128, 512], dt.float32, tag="t6")
            nc.scalar.activation(t6[:], u_sb[:], AF.Tanh)
            o6 = tailp.tile([128, 512], dt.float32, tag="o6")
            nc.vector.tensor_scalar_mul(o6[:], t6[:], C_CLIP)
            nc.sync.dma_start(out_p[:], o6[:])

    nc.compile()
    return nc


def _prep_inputs(encoder_output, query, Wq, bq, Wref, bref, v):
    bf16 = ml_dtypes.bfloat16
    # (K, B, E) -> (E, B, K), bf16
    enc_bf = np.asarray(encoder_output, np.float32).astype(bf16)
    encT = enc_bf.transpose(2, 1, 0)                   # (E, B, K) view

    def chunk4(w):                                     # (E, D) -> (4*128, 128)
        return np.ascontiguousarray(
            w.reshape(2, 128, 2, 128).transpose(0, 2, 1, 3).reshape(512, 128))

    def pack(w4):                                      # (4*128, X) -> (128, 4*X)
        x = w4.shape[1]
        return w4.reshape(4, 128, x).transpose(1, 0, 2).reshape(128, 4 * x)

    wref_p = pack(chunk4(np.asarray(Wref, np.float32).T))          # (128, 512)
    wq_p = pack(chunk4(np.asarray(Wq, np.float32).T))              # (128, 512)
    cbias = (np.asarray(bref, np.float32) + np.asarray(bq, np.float32))
    cbias_p = cbias.reshape(2, 128).T                               # (128, 2)
    v_p = np.asarray(v, np.float32).reshape(2, 128).T               # (128, 2)
    queryT = np.ascontiguousarray(np.asarray(query, np.float32).T)  # (E, B)

    ones_p = np.ones((128, 1), np.float32)
    cbf16 = np.concatenate([wref_p, v_p, ones_p], axis=1).astype(bf16)  # (128, 515)
    # f32 v columns appended to the f32 pack for the DVE per-partition mults

    in_maps = []
    for c in range(NCORES):
        enc_c = np.ascontiguousarray(encT[:, c * BL:(c + 1) * BL, :]).reshape(E, N)
        q_c = queryT[:, c * BL:(c + 1) * BL]                        # (256, 32)
        q_p = q_c.reshape(2, 128, BL).transpose(1, 0, 2).reshape(128, 2 * BL)
        cf32 = np.ascontiguousarray(np.concatenate(
            [wq_p, q_p, cbias_p, v_p], axis=1), dtype=np.float32)   # (128, 580)
        in_maps.append({
            "enc_t": enc_c,
            "cf32": cf32,
            "cbf16": cbf16,
        })
    return in_maps


def kernel(**inputs):
    global _compiled, last_exec_time_ns, last_results
    from concourse import bass_utils

    if _compiled is None:
        _compiled = _build()
    nc = _compiled

    in_maps = _prep_inputs(**inputs)
    res = bass_utils.run_bass_kernel_spmd(nc, in_maps, core_ids=list(range(NCORES)))
    last_exec_time_ns = res.exec_time_ns
    last_results = res
    # per-core (128, 512) f32 == row-major (32, 2048)
    out = np.concatenate(
        [r["out"].reshape(BL, K) for r in res.results], axis=0)
    return out


# revision 57
# speedup vs baseline: 1.3576x; 1.0166x over previous
"""Trainium2 Bass kernel for the attention-scoring module:

    q = query @ Wq.T + bq                               # (B, D)
    ref[b,d,k] = sum_e enc[k,b,e] * Wref[d,e] + bref[d]
    u[b,k] = sum_d v[d] * tanh(ref[b,d,k] + q[b,d])
    out = 10 * tanh(u)                                  # (B, K)

Data-parallel over batch: core c owns b in [32c, 32c+32).

Per-core dataflow (all big tensors bf16, f32 accumulation):
  - host pre-transposes enc to (E, b*K+k) so the contraction dim E lands
    on SBUF partitions with dense DMA.
  - main matmuls: psum[d(128), n(512)] += WrefT_chunk.T @ encT_chunk
  - bias (bref+bq+q_raw[b])[d] is per-partition in this layout -> folded
    into the ScalarE tanh activation for free.
  - the v-weighted d-reduction is a second-level matmul with stationary
    v (128,1): strips (1, 512) for the four k-blocks of one b land at
    partitions {0,32,64,96} of one PSUM bank via tile_position col
    groups (bank pre-zeroed, accumulation via start=False).
  - final 10*tanh(u) runs on the whole strip window (junk rows are
    free); the per-b output DMA plucks rows {0,32,64,96} with a
    stepped-partition access pattern.
"""

import os
import sys

import numpy as np

os.environ.setdefault("JAX_COMPILATION_CACHE_DIR", "/tmp/jaxcache")

for _p in ("/opt/trn_rl_repo", "/opt/pypackages"):
    if _p not in sys.path:
        sys.path.append(_p)

import ml_dtypes

E = 256
D = 256
K = 2048
B = 256
NCORES = 8
BL = B // NCORES          # 32 batch rows per core
N = BL * K                # 65536 flattened (b, k) per core
SLAB_B = 4                # b-rows per enc DMA slab
SLAB_N = SLAB_B * K       # 8192
C_CLIP = 10.0

_compiled = None
last_exec_time_ns = None
last_results = None


def _build():
    from concourse import bacc, bass, tile

    mybir = bass.mybir
    dt = mybir.dt
    AF = mybir.ActivationFunctionType

    nc = bacc.Bacc("TRN2", target_bir_lowering=False, debug=False,
                   num_devices=NCORES)

    enc_t = nc.declare_dram_parameter("enc_t", [E, N], dt.bfloat16, isOutput=False)
    # all small constants pre-packed host-side into two tensors so startup
    # is 2 DMAs, not 14 (each dma_start costs ~0.5us of queue issue latency)
    cf32_t = nc.declare_dram_parameter("cf32", [128, 580], dt.float32, isOutput=False)
    cbf16_t = nc.declare_dram_parameter("cbf16", [128, 515], dt.bfloat16, isOutput=False)
    out_p = nc.declare_dram_parameter("out", [128, 512], dt.float32, isOutput=True)

    with tile.TileContext(nc) as tc:
        with (
            tc.tile_pool(name="const", bufs=1) as constp,
            tc.tile_pool(name="enc", bufs=3) as encp,
            tc.tile_pool(name="tt", bufs=8) as tp,
            tc.tile_pool(name="tail", bufs=2) as tailp,
            tc.tile_pool(name="psum_m", bufs=3, space="PSUM") as pmp,
            tc.tile_pool(name="psum_s", bufs=2, space="PSUM") as psp,
        ):
            # ---- enc slab loading (slab 0 first, split per b-row so the
            # first matmuls start as early as possible) ----
            def alloc_slab(s):
                return [encp.tile([128, SLAB_N], dt.bfloat16, tag=f"enc{ec}",
                                  name=f"enc{ec}_s{s}")
                        for ec in range(2)]

            def emit_pieces(tiles, s, q0, q1, pieces=SLAB_B, dep=None):
                # ec-interleaved pieces q0..q1-1 of slab s; `dep` gates the
                # DMA issue so queued prefetches don't fair-share SDMA
                # bandwidth away from pieces that are needed right now
                w = SLAB_N // pieces
                for q in range(q0, q1):
                    for ec in range(2):
                        ins = nc.sync.dma_start(
                            tiles[ec][:, q * w:(q + 1) * w],
                            enc_t[ec * 128:(ec + 1) * 128,
                                  s * SLAB_N + q * w:s * SLAB_N + (q + 1) * w])
                        if dep is not None:
                            tile.add_dep_helper(ins.ins, dep.ins,
                                                reason="defer enc prefetch")

            def load_slab(s, pieces, dep=None):
                tiles = alloc_slab(s)
                emit_pieces(tiles, s, 0, pieces, pieces, dep=dep)
                return tiles

            # ---- constants: two packed DMAs ----
            cf32_sb = constp.tile([128, 580], dt.float32)
            cbf16_sb = constp.tile([128, 515], dt.bfloat16)
            bias_sb = constp.tile([128, 2 * BL], dt.float32)   # [:, dc*32 + b]
            u_sb = constp.tile([128, 512], dt.float32)         # [b*4+jj, kk]
            nc.sync.dma_start(cf32_sb[:], cf32_t[:])
            nc.sync.dma_start(cbf16_sb[:], cbf16_t[:])

            wq_sb = cf32_sb[:, 0:512]        # [:, (ec*2+dc)*128 + d]
            query_sb = cf32_sb[:, 512:576]   # [:, ec*32 + b]
            cbias_sb = cf32_sb[:, 576:578]
            v32_sb = cf32_sb[:, 578:580]     # f32 v for DVE per-partition mults
            wref_sb = cbf16_sb[:, 0:512]     # [:, (ec*2+dc)*128 + d]
            ones_sb = cbf16_sb[:, 514:515]

            # slab 0 staged: only b0's piece upfront, the rest fed into the
            # pipeline so the first matmuls aren't starved by SDMA
            # fair-sharing across all queued pieces
            slab0 = alloc_slab(0)
            emit_pieces(slab0, 0, 0, 1)

            # ---- q_rawT = (query @ Wq.T).T per d-chunk, + (bref + bq) ----
            for dc in range(2):
                qps = psp.tile([128, BL], dt.float32, tag="st")
                for ec in range(2):
                    nc.tensor.matmul(
                        qps[:],
                        wq_sb[:, (ec * 2 + dc) * 128:(ec * 2 + dc + 1) * 128],
                        query_sb[:, ec * BL:(ec + 1) * BL],
                        start=(ec == 0), stop=(ec == 1),
                    )
                nc.vector.tensor_scalar_add(bias_sb[:, dc * BL:(dc + 1) * BL],
                                            qps[:], cbias_sb[:, dc:dc + 1])

            # ---- main loop: the VectorE pre-combine runs in-round, the
            # ones-matmuls two rounds behind so TensorE never waits ----
            def emit_w(tts):
                # VectorE pre-combine, emitted early so the deferred
                # ones-matmuls never wait on the DVE queue
                w0 = tp.tile([128, 1024], dt.bfloat16, tag="w0", bufs=3)
                nc.vector.tensor_scalar_mul(w0[:], tts[0][:], v32_sb[:, 0:1])
                w1 = tp.tile([128, 1024], dt.bfloat16, tag="w1", bufs=3)
                nc.vector.tensor_scalar_mul(w1[:], tts[1][:], v32_sb[:, 1:2])
                w = tp.tile([128, 1024], dt.bfloat16, tag="w", bufs=3)
                nc.vector.tensor_add(w[:], w0[:], w1[:])
                return w

            def emit_strips(st4, w, b, kp):
                for kb in range(2):
                    jj = kp * 2 + kb
                    # start=True clears has_written per element, so the
                    # sibling strips in the same bank are unaffected
                    nc.tensor.matmul(
                        st4[32 * jj:32 * jj + 1, :],
                        ones_sb,
                        w[:, kb * 512:(kb + 1) * 512],
                        start=True, stop=True,
                        skip_group_check=True,
                        tile_position=(0, 32 * jj),
                    )
                if kp == 1:
                    # PSUM egress on DVE; a partition-strided SBUF->SBUF DMA
                    # compacts the 4 live rows into the dense u accumulator
                    sp = tailp.tile([128, 512], dt.float32, tag="sp")
                    nc.vector.tensor_copy(sp[:], st4[:])
                    nc.gpsimd.dma_start(u_sb[4 * b:4 * b + 4, :],
                                        sp[0:128:32, :])

            t6 = constp.tile([128, 512], dt.float32)
            o6 = constp.tile([128, 512], dt.float32)

            def emit_final(half):
                # out = 10 * tanh(u) for 16 b-rows (64 u rows)
                rows = slice(64 * half, 64 * half + 64)
                nc.scalar.activation(t6[rows, :], u_sb[rows, :], AF.Tanh)
                nc.vector.tensor_scalar_mul(o6[rows, :], t6[rows, :], C_CLIP)
                nc.sync.dma_start(out_p[rows, :], o6[rows, :])

            pend = []           # 2-deep deferral of the ones-matmuls
            NSLAB = N // SLAB_N
            cur_slab = slab0
            prev_mm = None      # first main matmul of the previous round
            for s in range(NSLAB):                  # 8 slabs of 4 b-rows
                nxt_slab = None
                for b_in in range(SLAB_B):
                    b = SLAB_B * s + b_in
                    st4 = psp.tile([128, 512], dt.float32, tag="st")
                    for kp in range(2):             # two 1024-wide n groups
                        if s == 0 and (b_in, kp) == (0, 1):
                            emit_pieces(cur_slab, 0, 1, 2, dep=prev_mm)
                        if s == 0 and (b_in, kp) == (1, 0):
                            emit_pieces(cur_slab, 0, 2, 4, dep=prev_mm)
                        pf_at = 2 if s == 0 else 1
                        if (b_in, kp) == (pf_at, 0) and s + 1 < NSLAB:
                            nxt_slab = load_slab(s + 1, pieces=2, dep=prev_mm)
                        first_mm = None
                        tts = []
                        for dc in range(2):
                            psd = pmp.tile([128, 1024], dt.float32, tag="psd")
                            # ec outer / kb inner: consecutive matmuls share
                            # the stationary operand -> fewer weight loads
                            for ec in range(2):
                                for kb in range(2):
                                    nseg = b_in * K + kp * 1024 + kb * 512
                                    ins = nc.tensor.matmul(
                                        psd[:, kb * 512:(kb + 1) * 512],
                                        wref_sb[:, (ec * 2 + dc) * 128:(ec * 2 + dc + 1) * 128],
                                        cur_slab[ec][:, nseg:nseg + 512],
                                        start=(ec == 0), stop=(ec == 1),
                                        skip_group_check=True,
                                    )
                                    if first_mm is None:
                                        first_mm = ins
                            ttile = tp.tile([128, 1024], dt.bfloat16, tag="tt")
                            nc.scalar.activation(
                                ttile[:], psd[:], AF.Tanh,
                                bias=bias_sb[:, dc * BL + b:dc * BL + b + 1],
                                scale=1.0)
                            tts.append(ttile)
                        w = emit_w(tts)
                        pend.append((st4, w, b, kp))
                        if len(pend) > 2:
                            emit_strips(*pend.pop(0))
                        prev_mm = first_mm
                        if (s, b_in, kp) == (4, 1, 0):
                            # b<=15 strips are all emitted by now; the
                            # first-half tail overlaps the second half of
                            # the main loop
                            emit_final(0)
                cur_slab = nxt_slab
            for args in pend:
                emit_strips(*args)
            emit_final(1)

    nc.compile()
    return nc


def _prep_inputs(encoder_output, query, Wq, bq, Wref, bref, v):
    bf16 = ml_dtypes.bfloat16
    # (K, B, E) -> (E, B, K), bf16
    enc_bf = np.asarray(encoder_output, np.float32).astype(bf16)
    encT = enc_bf.transpose(2, 1, 0)                   # (E, B, K) view

    def chunk4(w):                                     # (E, D) -> (4*128, 128)
        return np.ascontiguousarray(
            w.reshape(2, 128, 2, 128).transpose(0, 2, 1, 3).reshape(512, 128))

    def pack(w4):                                      # (4*128, X) -> (128, 4*X)
        x = w4.shape[1]
        return w4.reshape(4, 128, x).transpose(1, 0, 2).reshape(128, 4 * x)

    wref_p = pack(chunk4(np.asarray(Wref, np.float32).T))          # (128, 512)
    wq_p = pack(chunk4(np.asarray(Wq, np.float32).T))              # (128, 512)
    cbias = (np.asarray(bref, np.float32) + np.asarray(bq, np.float32))
    cbias_p = cbias.reshape(2, 128).T                               # (128, 2)
    v_p = np.asarray(v, np.float32).reshape(2, 128).T               # (128, 2)
    queryT = np.ascontiguousarray(np.asarray(query, np.float32).T)  # (E, B)

    ones_p = np.ones((128, 1), np.float32)
    cbf16 = np.concatenate([wref_p, v_p, ones_p], axis=1).astype(bf16)  # (128, 515)
    # f32 v columns appended to the f32 pack for the DVE per-partition mults

    in_maps = []
    for c in range(NCORES):
        enc_c = np.ascontiguousarray(encT[:, c * BL:(c + 1) * BL, :]).reshape(E, N)
        q_c = queryT[:, c * BL:(c + 1) * BL]                        # (256, 32)
        q_p = q_c.reshape(2, 128, BL).transpose(1, 0, 2).reshape(128, 2 * BL)
        cf32 = np.ascontiguousarray(np.concatenate(
            [wq_p, q_p, cbias_p, v_p], axis=1), dtype=np.float32)   # (128, 580)
        in_maps.append({
            "enc_t": enc_c,
            "cf32": cf32,
            "cbf16": cbf16,
        })
    return in_maps


def kernel(**inputs):
    global _compiled, last_exec_time_ns, last_results
    from concourse import bass_utils

    if _compiled is None:
        _compiled = _build()
    nc = _compiled

    in_maps = _prep_inputs(**inputs)
    res = bass_utils.run_bass_kernel_spmd(nc, in_maps, core_ids=list(range(NCORES)))
    last_exec_time_ns = res.exec_time_ns
    last_results = res
    # per-core (128, 512) f32 == row-major (32, 2048)
    out = np.concatenate(
        [r["out"].reshape(BL, K) for r in res.results], axis=0)
    return out
